# revision 18
# baseline (speedup 1.0000x reference)
"""Trainium2 Bass kernel for batch-8 multi-head self-attention with
contiguous-span masking (B=8, N=2048, DIN=DM=256, NH=4, DK=64).

Sharding: data-parallel over batch — core b computes sample b end-to-end.

Phase A (all-bf16 + j-compaction):
  - All matmul operands in bf16 (1 cyc/col streaming); host ships x, weights
    and mask rows pre-converted so no on-device casts are needed.
  - The K/V side of attention is compacted to the union-of-spans j-chunk
    window: the program is compiled (at first kernel() call) for
    NJC = max_b ceil(span_b / 128) j-chunks; each core receives its own
    span-aligned x slice (xKV).  Padding chunks carry a -1e10 vbias row so
    their exp() is exactly 0 -- same masking mechanism as in-span padding.
  - The uniform-row fix vector vbar = mean_j V_all = xbar @ Wv + bv is
    computed on device from the host-provided column-mean of x (mean
    commutes with the linear projection), replacing the on-device V-mean
    accumulation (which would be wrong under compaction: the reference's
    fp32 -1e10 absorption makes padding rows uniform over ALL 2048 keys).
  - kT/qT padded to 128 partitions (rows 66:128 zeroed) so LDWEIGHTS gets
    the compiler's fast-weight-load path (bf16, 128 rows).

Per-core dataflow (feature-on-partition, softmax reductions on free axis):
  S^T[j, i] = sum_d KT[d,j]*QT[d,i] + vbias_j*valid_i + NEG*inval_i
  P = exp(0.125 * S^T)   (no max subtraction: masked scores underflow to 0)
  U^T[d', i] = sum_j V_aug[j, d'] * P[j, i]  (row 64 = softmax denominator)
             + vbar_aug[d'] * inval_i        (uniform-row rank-1 fix)
  attT = U^T[0:64] / U^T[64];  outT = Wo^T attT + bo
"""

import numpy as np
import ml_dtypes

import concourse.bass as bass
import concourse.mybir as mybir
from concourse import bacc, bass_utils
from concourse.tile import TileContext


B, N, DIN, DM, NH, DK = 8, 2048, 256, 256, 4, 64
SCALE = 1.0 / 8.0  # 1/sqrt(DK)
NEG = -1e10

F32 = mybir.dt.float32
BF16 = mybir.dt.bfloat16
NPBF = ml_dtypes.bfloat16
IC = 512  # i-chunk width
NI = N // IC  # 4 i-chunks
DKP = DK + 2  # V_aug columns: 64 values + denominator ones + pad


def _emit(nc, tc, d, njc):
    Exp = mybir.ActivationFunctionType.Exp
    NKV = njc * 128

    with (
        tc.tile_pool(name="consts", bufs=1) as consts,
        tc.tile_pool(name="persist", bufs=1) as persist,
    ):
        # ---- persistent attention operands --------------------------------
        xT = [persist.tile([128, N], BF16, tag=f"xT{c}", name=f"xT{c}") for c in range(2)]
        xKV = [persist.tile([128, NKV], BF16, tag=f"xKV{c}", name=f"xKV{c}") for c in range(2)]
        qT = [persist.tile([128, N], BF16, tag=f"qT{h}", name=f"qT{h}") for h in range(NH)]
        kT = [persist.tile([128, NKV], BF16, tag=f"kT{h}", name=f"kT{h}") for h in range(NH)]
        vA = [persist.tile([128, NH, DKP], BF16, tag=f"vA{j}", name=f"vA{j}") for j in range(njc)]
        vbar = [consts.tile([1, DKP], BF16, tag=f"vbar{h}", name=f"vbar{h}") for h in range(NH)]

        wq, wk, wv, wo = [], [], [], []
        bqk, bo_sb, xbarT = [], [], []
        for c in range(2):
            for lst, name in ((wq, "Wq"), (wk, "Wk"), (wv, "Wv"), (wo, "Wo")):
                lst.append(consts.tile([128, DM], BF16, tag=f"{name}_r{c}", name=f"{name}_r{c}"))
            bqk.append(consts.tile([128, 2], F32, tag=f"bqk{c}", name=f"bqk{c}"))
            bo_sb.append(consts.tile([128, 1], F32, tag=f"bo{c}", name=f"bo{c}"))
            xbarT.append(consts.tile([128, 1], BF16, tag=f"xbarT{c}", name=f"xbarT{c}"))
        bv_r = consts.tile([1, DM], F32, tag="bv_r", name="bv_r")
        bv_bc = consts.tile([128, NH, DK], F32, tag="bv_bc", name="bv_bc")
        vpb = consts.tile([1, DM], F32, tag="vpb", name="vpb")
        inval_r = consts.tile([1, N], BF16, tag="inval_r", name="inval_r")
        vones = consts.tile([128, NH, 2], F32, tag="vones", name="vones")
        nc.vector.memset(vones, 1.0)

        # ---- direct DMA loads (dtypes match; critical-path order) ---------
        for c in range(2):
            nc.sync.dma_start(out=wk[c], in_=d["Wk"][c * 128 : (c + 1) * 128, :])
            nc.sync.dma_start(out=xKV[c], in_=d["xKV"][c * 128 : (c + 1) * 128, :])
        for c in range(2):
            nc.sync.dma_start(out=wq[c], in_=d["Wq"][c * 128 : (c + 1) * 128, :])
            nc.sync.dma_start(out=bqk[c], in_=d["bqk"][c * 128 : (c + 1) * 128, :])
            nc.sync.dma_start(out=xT[c], in_=d["xT"][c * 128 : (c + 1) * 128, :])
        qrows = consts.tile([2, N], BF16, tag="qrows", name="qrows")
        krows = consts.tile([2, NKV], BF16, tag="krows", name="krows")
        nc.sync.dma_start(out=qrows, in_=d["qrows"][:, :])
        nc.sync.dma_start(out=krows, in_=d["krows"][:, :])
        for c in range(2):
            nc.sync.dma_start(out=wv[c], in_=d["Wv"][c * 128 : (c + 1) * 128, :])
            nc.sync.dma_start(out=wo[c], in_=d["Wo"][c * 128 : (c + 1) * 128, :])
            nc.sync.dma_start(out=bo_sb[c], in_=d["bo"][c * 128 : (c + 1) * 128, :])
            nc.sync.dma_start(out=xbarT[c], in_=d["xbarT"][c * 128 : (c + 1) * 128, :])
        nc.sync.dma_start(out=bv_r, in_=d["bv"][0:1, :])
        nc.sync.dma_start(out=inval_r, in_=d["inval"][0:1, :])

        nc.gpsimd.partition_broadcast(
            bv_bc[:, :, :].rearrange("p h k -> p (h k)"), bv_r
        )
        # zero the 64:128 padding (FWL-friendly 128-row weights; partition
        # offsets must be 32-aligned), then write mask rows 64:66 on top
        for h in range(NH):
            nc.vector.memset(qT[h][64:128, :], 0.0)
            nc.vector.memset(kT[h][64:128, :], 0.0)
            nc.vector.tensor_copy(qT[h][64:66, :], qrows)
            nc.vector.tensor_copy(kT[h][64:66, :], krows)

        kvchunks = []  # (offset, width) pieces of the compacted K domain
        off = 0
        while off < NKV:
            w = min(512, NKV - off)
            kvchunks.append((off, w))
            off += w

        with (
            tc.tile_pool(name="psA", bufs=2, space="PSUM") as psA,
            tc.tile_pool(name="psS", bufs=3, space="PSUM") as psS,
            tc.tile_pool(name="expS", bufs=3) as expP,
            tc.tile_pool(name="nrm", bufs=3) as nrm,
            tc.tile_pool(name="attP", bufs=3) as attP,
            tc.tile_pool(name="outP", bufs=3) as outP,
        ):
            # ---- K then Q projections -------------------------------------
            def proj_kq(ws, src, sl, w, col, dst):
                for m in range(2):
                    p = psA.tile([128, IC], F32, tag="proj", name="proj")
                    for c in range(2):
                        nc.tensor.matmul(
                            p[:, 0:w],
                            lhsT=ws[c][:, m * 128 : (m + 1) * 128],
                            rhs=src[c][:, sl],
                            start=(c == 0),
                            stop=(c == 1),
                        )
                    for hh in range(2):
                        h = 2 * m + hh
                        nc.vector.tensor_scalar_add(
                            dst[h][0:64, sl],
                            p[hh * 64 : (hh + 1) * 64, 0:w],
                            bqk[m][hh * 64 : (hh + 1) * 64, col : col + 1],
                        )

            for off, w in kvchunks:
                proj_kq(wk, xKV, slice(off, off + w), w, 1, kT)
            for i in range(NI):
                proj_kq(wq, xT, bass.ts(i, IC), IC, 0, qT)
            for j in range(njc):
                p = psA.tile([128, DM], F32, tag="proj", name="proj")
                jsl = bass.ts(j, 128)
                for c in range(2):
                    nc.tensor.matmul(
                        p,
                        lhsT=xKV[c][:, jsl],
                        rhs=wv[c],
                        start=(c == 0),
                        stop=(c == 1),
                    )
                nc.vector.tensor_tensor(
                    vA[j][:, :, 0:DK],
                    p[:, :].rearrange("p (h k) -> p h k", h=NH),
                    bv_bc,
                    op=mybir.AluOpType.add,
                )
                nc.vector.tensor_copy(vA[j][:, :, DK:DKP], vones)

            # ---- vbar_aug = [xbar @ Wv + bv, 1.0] (uniform-row fix) -------
            vp = psA.tile([1, DM], F32, tag="proj", name="vbarp")
            for c in range(2):
                nc.tensor.matmul(
                    vp, lhsT=xbarT[c], rhs=wv[c], start=(c == 0), stop=(c == 1)
                )
            nc.vector.tensor_tensor(vpb, vp, bv_r, op=mybir.AluOpType.add)
            for h in range(NH):
                nc.vector.tensor_copy(vbar[h][0:1, 0:DK], vpb[0:1, h * 64 : (h + 1) * 64])
                nc.vector.memset(vbar[h][0:1, DK:DKP], 1.0)

            groups = [list(range(g, min(g + 2, njc))) for g in range(0, njc, 2)]

            # ---- attention + output projection ----------------------------
            def out_proj(i, attT):
                isl = bass.ts(i, IC)
                for e in range(2):
                    p = psA.tile([128, IC], F32, tag="proj", name="outp")
                    for c in range(2):
                        nc.tensor.matmul(
                            p,
                            lhsT=wo[c][:, e * 128 : (e + 1) * 128],
                            rhs=attT[c],
                            start=(c == 0),
                            stop=(c == 1),
                        )
                    o = outP.tile([128, IC], F32, tag="out", name="out")
                    nc.vector.tensor_scalar_add(o, p, bo_sb[e])
                    nc.sync.dma_start(
                        out=d["outT"][e * 128 : (e + 1) * 128, isl], in_=o
                    )

            pending = None
            for i in range(NI):
                isl = bass.ts(i, IC)
                attT = [attP.tile([128, IC], BF16, tag=f"attT{c}", name=f"attT{c}") for c in range(2)]
                for h in range(NH):
                    up = psA.tile([66, IC], F32, tag="proj", name="U")

                    def do_S(grp):
                        sp = psS.tile([128, 2, IC], F32, tag="S", name="S")
                        for gg, j in enumerate(grp):
                            nc.tensor.matmul(
                                sp[:, gg, :],
                                lhsT=kT[h][:, bass.ts(j, 128)],
                                rhs=qT[h][:, isl],
                                start=True,
                                stop=True,
                            )
                        return sp

                    sp = do_S(groups[0])
                    for gi, grp in enumerate(groups):
                        g = len(grp)
                        e = expP.tile([128, 2, IC], BF16, tag="expS", name="expS")
                        nc.scalar.activation(
                            e[:, 0:g, :], sp[:, 0:g, :], Exp, scale=SCALE
                        )
                        if gi + 1 < len(groups):
                            sp = do_S(groups[gi + 1])
                        for gg, j in enumerate(grp):
                            nc.tensor.matmul(
                                up,
                                lhsT=vA[j][:, h, :],
                                rhs=e[:, gg, :],
                                start=(j == 0),
                                stop=False,
                            )
                    nc.tensor.matmul(
                        up,
                        lhsT=vbar[h],
                        rhs=inval_r[0:1, isl],
                        start=False,
                        stop=True,
                    )
                    rsum = nrm.tile([1, IC], F32, tag="rsum", name="rsum")
                    nc.vector.tensor_copy(rsum, up[64:65, :])
                    rec = nrm.tile([1, IC], F32, tag="rec", name="rec")
                    nc.vector.reciprocal_approx_fast(rec, rsum)
                    bc = nrm.tile([64, IC], F32, tag="bc", name="bc")
                    nc.gpsimd.partition_broadcast(bc, rec[0:1, :])
                    nc.vector.tensor_mul(
                        attT[h // 2][(h % 2) * 64 : (h % 2 + 1) * 64, :],
                        up[0:64, :],
                        bc,
                    )
                if pending is not None:
                    out_proj(*pending)
                pending = (i, attT)
            out_proj(*pending)


_NC_CACHE = {}


def _build(njc):
    key = njc
    if key in _NC_CACHE:
        return _NC_CACHE[key]
    nc = bacc.Bacc("TRN2", debug=False, num_devices=B)
    NKV = njc * 128
    d = {
        "xT": nc.dram_tensor("xT", [DIN, N], BF16, kind="ExternalInput").ap(),
        "xKV": nc.dram_tensor("xKV", [DIN, NKV], BF16, kind="ExternalInput").ap(),
        "Wq": nc.dram_tensor("Wq", [DIN, DM], BF16, kind="ExternalInput").ap(),
        "Wk": nc.dram_tensor("Wk", [DIN, DM], BF16, kind="ExternalInput").ap(),
        "Wv": nc.dram_tensor("Wv", [DIN, DM], BF16, kind="ExternalInput").ap(),
        "Wo": nc.dram_tensor("Wo", [DM, DM], BF16, kind="ExternalInput").ap(),
        "bqk": nc.dram_tensor("bqk", [DM, 2], F32, kind="ExternalInput").ap(),
        "bv": nc.dram_tensor("bv", [1, DM], F32, kind="ExternalInput").ap(),
        "bo": nc.dram_tensor("bo", [DM, 1], F32, kind="ExternalInput").ap(),
        "xbarT": nc.dram_tensor("xbarT", [DIN, 1], BF16, kind="ExternalInput").ap(),
        "qrows": nc.dram_tensor("qrows", [2, N], BF16, kind="ExternalInput").ap(),
        "krows": nc.dram_tensor("krows", [2, NKV], BF16, kind="ExternalInput").ap(),
        "inval": nc.dram_tensor("inval", [1, N], BF16, kind="ExternalInput").ap(),
        "outT": nc.dram_tensor("outT", [DM, N], F32, kind="ExternalOutput").ap(),
    }
    with TileContext(nc) as tc:
        _emit(nc, tc, d, njc)
    nc.compile()
    _NC_CACHE[key] = nc
    return nc


def _host_marshal(x, attention_mask, Wq, bq, Wk, bk, Wv, bv, Wo, bo):
    x = np.asarray(x, dtype=np.float32)
    m = np.asarray(attention_mask).astype(bool)
    pos = np.arange(N)
    start = m.argmax(axis=1)  # first True index
    end = N - 1 - m[:, ::-1].argmax(axis=1)  # last True index (exclusive bound)
    valid = (pos[None, :] >= start[:, None]) & (pos[None, :] < end[:, None])
    valid_f = valid.astype(np.float32)
    vbias_f = np.where(valid, np.float32(0.0), np.float32(NEG)).astype(np.float32)

    A = (start // 128) * 128
    jc = np.ceil(end / 128.0).astype(np.int64) - A // 128
    njc = int(jc.max())
    W = njc * 128

    common = {
        "Wq": np.ascontiguousarray(Wq, dtype=np.float32).astype(NPBF),
        "Wk": np.ascontiguousarray(Wk, dtype=np.float32).astype(NPBF),
        "Wv": np.ascontiguousarray(Wv, dtype=np.float32).astype(NPBF),
        "Wo": np.ascontiguousarray(Wo, dtype=np.float32).astype(NPBF),
        "bqk": np.ascontiguousarray(
            np.stack([np.asarray(bq), np.asarray(bk)], axis=1), dtype=np.float32
        ),
        "bv": np.asarray(bv, dtype=np.float32).reshape(1, DM),
        "bo": np.asarray(bo, dtype=np.float32).reshape(DM, 1),
    }
    in_maps = []
    for b in range(B):
        im = dict(common)
        xTb = np.ascontiguousarray(x[b].T).astype(NPBF)
        im["xT"] = xTb
        a = int(A[b])
        avail = min(N, a + W) - a
        xkv = np.zeros((DIN, W), dtype=NPBF)
        xkv[:, 0:avail] = xTb[:, a : a + avail]
        im["xKV"] = xkv
        im["xbarT"] = x[b].mean(axis=0).reshape(DIN, 1).astype(NPBF)
        inval = np.float32(1.0) - valid_f[b : b + 1]
        im["qrows"] = np.concatenate([valid_f[b : b + 1], inval], axis=0).astype(NPBF)
        kr = np.full((2, W), NEG, dtype=np.float32)
        kr[0, 0:avail] = vbias_f[b, a : a + avail]
        im["krows"] = kr.astype(NPBF)
        im["inval"] = inval.astype(NPBF)
        in_maps.append(im)
    return in_maps, njc


def kernel(x, attention_mask, Wq, bq, Wk, bk, Wv, bv, Wo, bo, _trace=False):
    in_maps, njc = _host_marshal(x, attention_mask, Wq, bq, Wk, bk, Wv, bv, Wo, bo)
    nc = _build(njc)
    res = bass_utils.run_bass_kernel_spmd(
        nc, in_maps, core_ids=list(range(B)), trace=_trace
    )
    out = np.stack([np.ascontiguousarray(r["outT"].T) for r in res.results], axis=0)
    if _trace:
        kernel.last_exec_time_ns = res.exec_time_ns
        kernel.last_results = res
    return out


# revision 19
# speedup vs baseline: 1.0336x; 1.0336x over previous
"""Trainium2 Bass kernel for batch-8 multi-head self-attention with
contiguous-span masking (B=8, N=2048, DIN=DM=256, NH=4, DK=64).

Sharding: data-parallel over batch — core b computes sample b end-to-end.

Phase A (all-bf16 + j-compaction):
  - All matmul operands in bf16 (1 cyc/col streaming); host ships x, weights
    and mask rows pre-converted so no on-device casts are needed.
  - The K/V side of attention is compacted to the union-of-spans j-chunk
    window: the program is compiled (at first kernel() call) for
    NJC = max_b ceil(span_b / 128) j-chunks; each core receives its own
    span-aligned x slice (xKV).  Padding chunks carry a -1e10 vbias row so
    their exp() is exactly 0 -- same masking mechanism as in-span padding.
  - The uniform-row fix vector vbar = mean_j V_all = xbar @ Wv + bv is
    computed on device from the host-provided column-mean of x (mean
    commutes with the linear projection), replacing the on-device V-mean
    accumulation (which would be wrong under compaction: the reference's
    fp32 -1e10 absorption makes padding rows uniform over ALL 2048 keys).
  - kT/qT padded to 128 partitions (rows 66:128 zeroed) so LDWEIGHTS gets
    the compiler's fast-weight-load path (bf16, 128 rows).

Per-core dataflow (feature-on-partition, softmax reductions on free axis):
  S^T[j, i] = sum_d KT[d,j]*QT[d,i] + vbias_j*valid_i + NEG*inval_i
  P = exp(0.125 * S^T)   (no max subtraction: masked scores underflow to 0)
  U^T[d', i] = sum_j V_aug[j, d'] * P[j, i]  (row 64 = softmax denominator)
             + vbar_aug[d'] * inval_i        (uniform-row rank-1 fix)
  attT = U^T[0:64] / U^T[64];  outT = Wo^T attT + bo
"""

import numpy as np
import ml_dtypes

import concourse.bass as bass
import concourse.mybir as mybir
from concourse import bacc, bass_utils
from concourse.tile import TileContext


B, N, DIN, DM, NH, DK = 8, 2048, 256, 256, 4, 64
SCALE = 1.0 / 8.0  # 1/sqrt(DK)
NEG = -1e10

F32 = mybir.dt.float32
BF16 = mybir.dt.bfloat16
NPBF = ml_dtypes.bfloat16
IC = 512  # i-chunk width
NI = N // IC  # 4 i-chunks
DKP = DK + 2  # V_aug columns: 64 values + denominator ones + pad


def _emit(nc, tc, d, njc):
    Exp = mybir.ActivationFunctionType.Exp
    NKV = njc * 128

    with (
        tc.tile_pool(name="consts", bufs=1) as consts,
        tc.tile_pool(name="persist", bufs=1) as persist,
    ):
        # ---- persistent attention operands --------------------------------
        xT = [persist.tile([128, N], BF16, tag=f"xT{c}", name=f"xT{c}") for c in range(2)]
        xKV = [persist.tile([128, NKV], BF16, tag=f"xKV{c}", name=f"xKV{c}") for c in range(2)]
        qT = [persist.tile([128, N], BF16, tag=f"qT{h}", name=f"qT{h}") for h in range(NH)]
        kT = [persist.tile([128, NKV], BF16, tag=f"kT{h}", name=f"kT{h}") for h in range(NH)]
        vA = [persist.tile([128, NH, DKP], BF16, tag=f"vA{j}", name=f"vA{j}") for j in range(njc)]
        vbar = [consts.tile([1, DKP], BF16, tag=f"vbar{h}", name=f"vbar{h}") for h in range(NH)]

        wq, wk, wv, wo = [], [], [], []
        bqk, bo_sb, xbarT = [], [], []
        for c in range(2):
            for lst, name in ((wq, "Wq"), (wk, "Wk"), (wv, "Wv"), (wo, "Wo")):
                lst.append(consts.tile([128, DM], BF16, tag=f"{name}_r{c}", name=f"{name}_r{c}"))
            bqk.append(consts.tile([128, 2], F32, tag=f"bqk{c}", name=f"bqk{c}"))
            bo_sb.append(consts.tile([128, 1], F32, tag=f"bo{c}", name=f"bo{c}"))
            xbarT.append(consts.tile([128, 1], BF16, tag=f"xbarT{c}", name=f"xbarT{c}"))
        bv_r = consts.tile([1, DM], F32, tag="bv_r", name="bv_r")
        bv_bc = consts.tile([128, NH, DK], F32, tag="bv_bc", name="bv_bc")
        vpb = consts.tile([1, DM], F32, tag="vpb", name="vpb")
        inval_r = consts.tile([1, N], BF16, tag="inval_r", name="inval_r")
        vones = consts.tile([128, NH, 2], F32, tag="vones", name="vones")
        nc.vector.memset(vones, 1.0)

        # ---- direct DMA loads (dtypes match; critical-path order) ---------
        for c in range(2):
            nc.sync.dma_start(out=wk[c], in_=d["Wk"][c * 128 : (c + 1) * 128, :])
            nc.sync.dma_start(out=xKV[c], in_=d["xKV"][c * 128 : (c + 1) * 128, :])
        for c in range(2):
            nc.sync.dma_start(out=wq[c], in_=d["Wq"][c * 128 : (c + 1) * 128, :])
            nc.sync.dma_start(out=bqk[c], in_=d["bqk"][c * 128 : (c + 1) * 128, :])
            nc.sync.dma_start(out=xT[c], in_=d["xT"][c * 128 : (c + 1) * 128, :])
        qrows = consts.tile([2, N], BF16, tag="qrows", name="qrows")
        krows = consts.tile([2, NKV], BF16, tag="krows", name="krows")
        nc.sync.dma_start(out=qrows, in_=d["qrows"][:, :])
        nc.sync.dma_start(out=krows, in_=d["krows"][:, :])
        for c in range(2):
            nc.sync.dma_start(out=wv[c], in_=d["Wv"][c * 128 : (c + 1) * 128, :])
            nc.sync.dma_start(out=wo[c], in_=d["Wo"][c * 128 : (c + 1) * 128, :])
            nc.sync.dma_start(out=bo_sb[c], in_=d["bo"][c * 128 : (c + 1) * 128, :])
            nc.sync.dma_start(out=xbarT[c], in_=d["xbarT"][c * 128 : (c + 1) * 128, :])
        nc.sync.dma_start(out=bv_r, in_=d["bv"][0:1, :])
        nc.sync.dma_start(out=inval_r, in_=d["inval"][0:1, :])

        nc.gpsimd.partition_broadcast(
            bv_bc[:, :, :].rearrange("p h k -> p (h k)"), bv_r
        )
        # zero the 64:128 padding (FWL-friendly 128-row weights; partition
        # offsets must be 32-aligned), then write mask rows 64:66 on top
        for h in range(NH):
            nc.vector.memset(qT[h][64:128, :], 0.0)
            nc.vector.memset(kT[h][64:128, :], 0.0)
            nc.vector.tensor_copy(qT[h][64:66, :], qrows)
            nc.vector.tensor_copy(kT[h][64:66, :], krows)

        kvchunks = []  # (offset, width) pieces of the compacted K domain
        off = 0
        while off < NKV:
            w = min(512, NKV - off)
            kvchunks.append((off, w))
            off += w

        with (
            tc.tile_pool(name="psA", bufs=2, space="PSUM") as psA,
            tc.tile_pool(name="psS", bufs=3, space="PSUM") as psS,
            tc.tile_pool(name="expS", bufs=3) as expP,
            tc.tile_pool(name="nrm", bufs=3) as nrm,
            tc.tile_pool(name="attP", bufs=3) as attP,
            tc.tile_pool(name="outP", bufs=3) as outP,
        ):
            # ---- K then Q projections -------------------------------------
            def proj_kq(ws, src, sl, w, col, dst):
                for m in range(2):
                    p = psA.tile([128, IC], F32, tag="proj", name="proj")
                    for c in range(2):
                        nc.tensor.matmul(
                            p[:, 0:w],
                            lhsT=ws[c][:, m * 128 : (m + 1) * 128],
                            rhs=src[c][:, sl],
                            start=(c == 0),
                            stop=(c == 1),
                        )
                    for hh in range(2):
                        h = 2 * m + hh
                        if hh:
                            nc.scalar.activation(
                                dst[h][0:64, sl],
                                p[hh * 64 : (hh + 1) * 64, 0:w],
                                mybir.ActivationFunctionType.Identity,
                                bias=bqk[m][hh * 64 : (hh + 1) * 64, col : col + 1],
                            )
                        else:
                            nc.vector.tensor_scalar_add(
                                dst[h][0:64, sl],
                                p[hh * 64 : (hh + 1) * 64, 0:w],
                                bqk[m][hh * 64 : (hh + 1) * 64, col : col + 1],
                            )

            for off, w in kvchunks:
                proj_kq(wk, xKV, slice(off, off + w), w, 1, kT)
            for i in range(NI):
                proj_kq(wq, xT, bass.ts(i, IC), IC, 0, qT)
            for j in range(njc):
                p = psA.tile([128, DM], F32, tag="proj", name="proj")
                jsl = bass.ts(j, 128)
                for c in range(2):
                    nc.tensor.matmul(
                        p,
                        lhsT=xKV[c][:, jsl],
                        rhs=wv[c],
                        start=(c == 0),
                        stop=(c == 1),
                    )
                nc.vector.tensor_tensor(
                    vA[j][:, :, 0:DK],
                    p[:, :].rearrange("p (h k) -> p h k", h=NH),
                    bv_bc,
                    op=mybir.AluOpType.add,
                )
                nc.vector.tensor_copy(vA[j][:, :, DK:DKP], vones)

            # ---- vbar_aug = [xbar @ Wv + bv, 1.0] (uniform-row fix) -------
            vp = psA.tile([1, DM], F32, tag="proj", name="vbarp")
            for c in range(2):
                nc.tensor.matmul(
                    vp, lhsT=xbarT[c], rhs=wv[c], start=(c == 0), stop=(c == 1)
                )
            nc.vector.tensor_tensor(vpb, vp, bv_r, op=mybir.AluOpType.add)
            for h in range(NH):
                nc.vector.tensor_copy(vbar[h][0:1, 0:DK], vpb[0:1, h * 64 : (h + 1) * 64])
                nc.vector.memset(vbar[h][0:1, DK:DKP], 1.0)

            groups = [list(range(g, min(g + 2, njc))) for g in range(0, njc, 2)]

            # ---- attention + output projection ----------------------------
            def out_proj(i, attT):
                isl = bass.ts(i, IC)
                for e in range(2):
                    p = psA.tile([128, IC], F32, tag="proj", name="outp")
                    for c in range(2):
                        nc.tensor.matmul(
                            p,
                            lhsT=wo[c][:, e * 128 : (e + 1) * 128],
                            rhs=attT[c],
                            start=(c == 0),
                            stop=(c == 1),
                        )
                    o = outP.tile([128, IC], F32, tag="out", name="out")
                    nc.vector.tensor_scalar_add(o, p, bo_sb[e])
                    nc.sync.dma_start(
                        out=d["outT"][e * 128 : (e + 1) * 128, isl], in_=o
                    )

            pending = None
            for i in range(NI):
                isl = bass.ts(i, IC)
                attT = [attP.tile([128, IC], BF16, tag=f"attT{c}", name=f"attT{c}") for c in range(2)]
                for h in range(NH):
                    up = psA.tile([66, IC], F32, tag="proj", name="U")
                    for grp in groups:
                        g = len(grp)
                        sp = psS.tile([128, 2, IC], F32, tag="S", name="S")
                        for gg, j in enumerate(grp):
                            nc.tensor.matmul(
                                sp[:, gg, :],
                                lhsT=kT[h][:, bass.ts(j, 128)],
                                rhs=qT[h][:, isl],
                                start=True,
                                stop=True,
                            )
                        e = expP.tile([128, 2, IC], BF16, tag="expS", name="expS")
                        nc.scalar.activation(
                            e[:, 0:g, :], sp[:, 0:g, :], Exp, scale=SCALE
                        )
                        for gg, j in enumerate(grp):
                            nc.tensor.matmul(
                                up,
                                lhsT=vA[j][:, h, :],
                                rhs=e[:, gg, :],
                                start=(j == 0),
                                stop=False,
                            )
                    nc.tensor.matmul(
                        up,
                        lhsT=vbar[h],
                        rhs=inval_r[0:1, isl],
                        start=False,
                        stop=True,
                    )
                    rsum = nrm.tile([1, IC], F32, tag="rsum", name="rsum")
                    nc.vector.tensor_copy(rsum, up[64:65, :])
                    rec = nrm.tile([1, IC], F32, tag="rec", name="rec")
                    nc.vector.reciprocal_approx_fast(rec, rsum)
                    bc = nrm.tile([64, IC], F32, tag="bc", name="bc")
                    nc.gpsimd.partition_broadcast(bc, rec[0:1, :])
                    nc.vector.tensor_mul(
                        attT[h // 2][(h % 2) * 64 : (h % 2 + 1) * 64, :],
                        up[0:64, :],
                        bc,
                    )
                if pending is not None:
                    out_proj(*pending)
                pending = (i, attT)
            out_proj(*pending)


_NC_CACHE = {}


def _build(njc):
    key = njc
    if key in _NC_CACHE:
        return _NC_CACHE[key]
    nc = bacc.Bacc("TRN2", debug=False, num_devices=B)
    NKV = njc * 128
    d = {
        "xT": nc.dram_tensor("xT", [DIN, N], BF16, kind="ExternalInput").ap(),
        "xKV": nc.dram_tensor("xKV", [DIN, NKV], BF16, kind="ExternalInput").ap(),
        "Wq": nc.dram_tensor("Wq", [DIN, DM], BF16, kind="ExternalInput").ap(),
        "Wk": nc.dram_tensor("Wk", [DIN, DM], BF16, kind="ExternalInput").ap(),
        "Wv": nc.dram_tensor("Wv", [DIN, DM], BF16, kind="ExternalInput").ap(),
        "Wo": nc.dram_tensor("Wo", [DM, DM], BF16, kind="ExternalInput").ap(),
        "bqk": nc.dram_tensor("bqk", [DM, 2], F32, kind="ExternalInput").ap(),
        "bv": nc.dram_tensor("bv", [1, DM], F32, kind="ExternalInput").ap(),
        "bo": nc.dram_tensor("bo", [DM, 1], F32, kind="ExternalInput").ap(),
        "xbarT": nc.dram_tensor("xbarT", [DIN, 1], BF16, kind="ExternalInput").ap(),
        "qrows": nc.dram_tensor("qrows", [2, N], BF16, kind="ExternalInput").ap(),
        "krows": nc.dram_tensor("krows", [2, NKV], BF16, kind="ExternalInput").ap(),
        "inval": nc.dram_tensor("inval", [1, N], BF16, kind="ExternalInput").ap(),
        "outT": nc.dram_tensor("outT", [DM, N], F32, kind="ExternalOutput").ap(),
    }
    with TileContext(nc) as tc:
        _emit(nc, tc, d, njc)
    nc.compile()
    _NC_CACHE[key] = nc
    return nc


def _host_marshal(x, attention_mask, Wq, bq, Wk, bk, Wv, bv, Wo, bo):
    x = np.asarray(x, dtype=np.float32)
    m = np.asarray(attention_mask).astype(bool)
    pos = np.arange(N)
    start = m.argmax(axis=1)  # first True index
    end = N - 1 - m[:, ::-1].argmax(axis=1)  # last True index (exclusive bound)
    valid = (pos[None, :] >= start[:, None]) & (pos[None, :] < end[:, None])
    valid_f = valid.astype(np.float32)
    vbias_f = np.where(valid, np.float32(0.0), np.float32(NEG)).astype(np.float32)

    A = (start // 128) * 128
    jc = np.ceil(end / 128.0).astype(np.int64) - A // 128
    njc = int(jc.max())
    W = njc * 128

    common = {
        "Wq": np.ascontiguousarray(Wq, dtype=np.float32).astype(NPBF),
        "Wk": np.ascontiguousarray(Wk, dtype=np.float32).astype(NPBF),
        "Wv": np.ascontiguousarray(Wv, dtype=np.float32).astype(NPBF),
        "Wo": np.ascontiguousarray(Wo, dtype=np.float32).astype(NPBF),
        "bqk": np.ascontiguousarray(
            np.stack([np.asarray(bq), np.asarray(bk)], axis=1), dtype=np.float32
        ),
        "bv": np.asarray(bv, dtype=np.float32).reshape(1, DM),
        "bo": np.asarray(bo, dtype=np.float32).reshape(DM, 1),
    }
    in_maps = []
    for b in range(B):
        im = dict(common)
        xTb = np.ascontiguousarray(x[b].T).astype(NPBF)
        im["xT"] = xTb
        a = int(A[b])
        avail = min(N, a + W) - a
        xkv = np.zeros((DIN, W), dtype=NPBF)
        xkv[:, 0:avail] = xTb[:, a : a + avail]
        im["xKV"] = xkv
        im["xbarT"] = x[b].mean(axis=0).reshape(DIN, 1).astype(NPBF)
        inval = np.float32(1.0) - valid_f[b : b + 1]
        im["qrows"] = np.concatenate([valid_f[b : b + 1], inval], axis=0).astype(NPBF)
        kr = np.full((2, W), NEG, dtype=np.float32)
        kr[0, 0:avail] = vbias_f[b, a : a + avail]
        im["krows"] = kr.astype(NPBF)
        im["inval"] = inval.astype(NPBF)
        in_maps.append(im)
    return in_maps, njc


def kernel(x, attention_mask, Wq, bq, Wk, bk, Wv, bv, Wo, bo, _trace=False):
    in_maps, njc = _host_marshal(x, attention_mask, Wq, bq, Wk, bk, Wv, bv, Wo, bo)
    nc = _build(njc)
    res = bass_utils.run_bass_kernel_spmd(
        nc, in_maps, core_ids=list(range(B)), trace=_trace
    )
    out = np.stack([np.ascontiguousarray(r["outT"].T) for r in res.results], axis=0)
    if _trace:
        kernel.last_exec_time_ns = res.exec_time_ns
        kernel.last_results = res
    return out


# revision 20
# speedup vs baseline: 1.0564x; 1.0221x over previous
"""Trainium2 Bass kernel for batch-8 multi-head self-attention with
contiguous-span masking (B=8, N=2048, DIN=DM=256, NH=4, DK=64).

Sharding: data-parallel over batch — core b computes sample b end-to-end.

Phase A (all-bf16 + j-compaction):
  - All matmul operands in bf16 (1 cyc/col streaming); host ships x, weights
    and mask rows pre-converted so no on-device casts are needed.
  - The K/V side of attention is compacted to the union-of-spans j-chunk
    window: the program is compiled (at first kernel() call) for
    NJC = max_b ceil(span_b / 128) j-chunks; each core receives its own
    span-aligned x slice (xKV).  Padding chunks carry a -1e10 vbias row so
    their exp() is exactly 0 -- same masking mechanism as in-span padding.
  - The uniform-row fix vector vbar = mean_j V_all = xbar @ Wv + bv is
    computed on device from the host-provided column-mean of x (mean
    commutes with the linear projection), replacing the on-device V-mean
    accumulation (which would be wrong under compaction: the reference's
    fp32 -1e10 absorption makes padding rows uniform over ALL 2048 keys).
  - kT/qT padded to 128 partitions (rows 66:128 zeroed) so LDWEIGHTS gets
    the compiler's fast-weight-load path (bf16, 128 rows).

Per-core dataflow (feature-on-partition, softmax reductions on free axis):
  S^T[j, i] = sum_d KT[d,j]*QT[d,i] + vbias_j*valid_i + NEG*inval_i
  P = exp(0.125 * S^T)   (no max subtraction: masked scores underflow to 0)
  U^T[d', i] = sum_j V_aug[j, d'] * P[j, i]  (row 64 = softmax denominator)
             + vbar_aug[d'] * inval_i        (uniform-row rank-1 fix)
  attT = U^T[0:64] / U^T[64];  outT = Wo^T attT + bo
"""

import numpy as np
import ml_dtypes

import concourse.bass as bass
import concourse.mybir as mybir
from concourse import bacc, bass_utils
from concourse.tile import TileContext


B, N, DIN, DM, NH, DK = 8, 2048, 256, 256, 4, 64
SCALE = 1.0 / 8.0  # 1/sqrt(DK)
NEG = -1e10

F32 = mybir.dt.float32
BF16 = mybir.dt.bfloat16
NPBF = ml_dtypes.bfloat16
IC = 512  # i-chunk width
NI = N // IC  # 4 i-chunks
DKP = DK + 2  # V_aug columns: 64 values + denominator ones + pad


def _emit(nc, tc, d, njc):
    Exp = mybir.ActivationFunctionType.Exp
    NKV = njc * 128

    with (
        tc.tile_pool(name="consts", bufs=1) as consts,
        tc.tile_pool(name="persist", bufs=1) as persist,
    ):
        # ---- persistent attention operands --------------------------------
        xT = [persist.tile([128, N], BF16, tag=f"xT{c}", name=f"xT{c}") for c in range(2)]
        xKV = [persist.tile([128, NKV], BF16, tag=f"xKV{c}", name=f"xKV{c}") for c in range(2)]
        qT = [persist.tile([66, N], BF16, tag=f"qT{h}", name=f"qT{h}") for h in range(NH)]
        kT = [persist.tile([66, NKV], BF16, tag=f"kT{h}", name=f"kT{h}") for h in range(NH)]
        vA = [persist.tile([128, NH, DKP], BF16, tag=f"vA{j}", name=f"vA{j}") for j in range(njc)]
        vbar = [consts.tile([1, DKP], BF16, tag=f"vbar{h}", name=f"vbar{h}") for h in range(NH)]

        wq, wk, wv, wo = [], [], [], []
        bqk, bo_sb, xbarT = [], [], []
        for c in range(2):
            for lst, name in ((wq, "Wq"), (wk, "Wk"), (wv, "Wv"), (wo, "Wo")):
                lst.append(consts.tile([128, DM], BF16, tag=f"{name}_r{c}", name=f"{name}_r{c}"))
            bqk.append(consts.tile([128, 2], F32, tag=f"bqk{c}", name=f"bqk{c}"))
            bo_sb.append(consts.tile([128, 1], F32, tag=f"bo{c}", name=f"bo{c}"))
            xbarT.append(consts.tile([128, 1], BF16, tag=f"xbarT{c}", name=f"xbarT{c}"))
        bv_r = consts.tile([1, DM], F32, tag="bv_r", name="bv_r")
        bv_bc = consts.tile([128, NH, DK], F32, tag="bv_bc", name="bv_bc")
        vpb = consts.tile([1, DM], F32, tag="vpb", name="vpb")
        inval_r = consts.tile([1, N], BF16, tag="inval_r", name="inval_r")
        vones = consts.tile([128, NH, 2], F32, tag="vones", name="vones")
        nc.vector.memset(vones, 1.0)

        # ---- direct DMA loads; mask rows go straight into kT/qT rows
        # 64:66 (no staging copies); x loads chunked so projections start early
        for c in range(2):
            nc.sync.dma_start(out=wk[c], in_=d["Wk"][c * 128 : (c + 1) * 128, :])
        for off in range(0, NKV, 512):
            w = min(512, NKV - off)
            for c in range(2):
                nc.sync.dma_start(
                    out=xKV[c][:, off : off + w],
                    in_=d["xKV"][c * 128 : (c + 1) * 128, off : off + w],
                )
        for h in range(NH):
            nc.sync.dma_start(out=kT[h][64:66, :], in_=d["krows"][:, :])
        for c in range(2):
            nc.sync.dma_start(out=wq[c], in_=d["Wq"][c * 128 : (c + 1) * 128, :])
            nc.sync.dma_start(out=bqk[c], in_=d["bqk"][c * 128 : (c + 1) * 128, :])
            nc.sync.dma_start(
                out=xT[c][:, 0:IC], in_=d["xT"][c * 128 : (c + 1) * 128, 0:IC]
            )
        for h in range(NH):
            nc.sync.dma_start(out=qT[h][64:66, :], in_=d["qrows"][:, :])
        nc.sync.dma_start(out=inval_r, in_=d["inval"][0:1, :])
        for c in range(2):
            nc.sync.dma_start(out=xbarT[c], in_=d["xbarT"][c * 128 : (c + 1) * 128, :])
            nc.sync.dma_start(out=wv[c], in_=d["Wv"][c * 128 : (c + 1) * 128, :])
        for i in range(1, NI):
            for c in range(2):
                nc.sync.dma_start(
                    out=xT[c][:, bass.ts(i, IC)],
                    in_=d["xT"][c * 128 : (c + 1) * 128, bass.ts(i, IC)],
                )
        for c in range(2):
            nc.sync.dma_start(out=wo[c], in_=d["Wo"][c * 128 : (c + 1) * 128, :])
            nc.sync.dma_start(out=bo_sb[c], in_=d["bo"][c * 128 : (c + 1) * 128, :])
        nc.sync.dma_start(out=bv_r, in_=d["bv"][0:1, :])

        nc.gpsimd.partition_broadcast(
            bv_bc[:, :, :].rearrange("p h k -> p (h k)"), bv_r
        )

        kvchunks = []  # (offset, width) pieces of the compacted K domain
        off = 0
        while off < NKV:
            w = min(512, NKV - off)
            kvchunks.append((off, w))
            off += w

        with (
            tc.tile_pool(name="psA", bufs=2, space="PSUM") as psA,
            tc.tile_pool(name="psS", bufs=3, space="PSUM") as psS,
            tc.tile_pool(name="expS", bufs=3) as expP,
            tc.tile_pool(name="nrm", bufs=3) as nrm,
            tc.tile_pool(name="attP", bufs=3) as attP,
            tc.tile_pool(name="outP", bufs=3) as outP,
        ):
            # ---- K then Q projections -------------------------------------
            def proj_kq(ws, src, sl, w, col, dst):
                for m in range(2):
                    p = psA.tile([128, IC], F32, tag="proj", name="proj")
                    for c in range(2):
                        nc.tensor.matmul(
                            p[:, 0:w],
                            lhsT=ws[c][:, m * 128 : (m + 1) * 128],
                            rhs=src[c][:, sl],
                            start=(c == 0),
                            stop=(c == 1),
                        )
                    for hh in range(2):
                        h = 2 * m + hh
                        nc.vector.tensor_scalar_add(
                            dst[h][0:64, sl],
                            p[hh * 64 : (hh + 1) * 64, 0:w],
                            bqk[m][hh * 64 : (hh + 1) * 64, col : col + 1],
                        )

            for off, w in kvchunks:
                proj_kq(wk, xKV, slice(off, off + w), w, 1, kT)
            proj_kq(wq, xT, bass.ts(0, IC), IC, 0, qT)
            for j in range(njc):
                p = psA.tile([128, DM], F32, tag="proj", name="proj")
                jsl = bass.ts(j, 128)
                for c in range(2):
                    nc.tensor.matmul(
                        p,
                        lhsT=xKV[c][:, jsl],
                        rhs=wv[c],
                        start=(c == 0),
                        stop=(c == 1),
                    )
                nc.vector.tensor_tensor(
                    vA[j][:, :, 0:DK],
                    p[:, :].rearrange("p (h k) -> p h k", h=NH),
                    bv_bc,
                    op=mybir.AluOpType.add,
                )
                nc.vector.tensor_copy(vA[j][:, :, DK:DKP], vones)

            # ---- vbar_aug = [xbar @ Wv + bv, 1.0] (uniform-row fix) -------
            vp = psA.tile([1, DM], F32, tag="proj", name="vbarp")
            for c in range(2):
                nc.tensor.matmul(
                    vp, lhsT=xbarT[c], rhs=wv[c], start=(c == 0), stop=(c == 1)
                )
            nc.vector.tensor_tensor(vpb, vp, bv_r, op=mybir.AluOpType.add)
            for h in range(NH):
                nc.vector.tensor_copy(vbar[h][0:1, 0:DK], vpb[0:1, h * 64 : (h + 1) * 64])
                nc.vector.memset(vbar[h][0:1, DK:DKP], 1.0)

            groups = [list(range(g, min(g + 2, njc))) for g in range(0, njc, 2)]

            # ---- attention + output projection ----------------------------
            def out_proj(i, attT):
                isl = bass.ts(i, IC)
                for e in range(2):
                    p = psA.tile([128, IC], F32, tag="proj", name="outp")
                    for c in range(2):
                        nc.tensor.matmul(
                            p,
                            lhsT=wo[c][:, e * 128 : (e + 1) * 128],
                            rhs=attT[c],
                            start=(c == 0),
                            stop=(c == 1),
                        )
                    o = outP.tile([128, IC], F32, tag="out", name="out")
                    nc.vector.tensor_scalar_add(o, p, bo_sb[e])
                    nc.sync.dma_start(
                        out=d["outT"][e * 128 : (e + 1) * 128, isl], in_=o
                    )

            pending = None
            for i in range(NI):
                if i + 1 < NI:
                    proj_kq(wq, xT, bass.ts(i + 1, IC), IC, 0, qT)
                isl = bass.ts(i, IC)
                attT = [attP.tile([128, IC], BF16, tag=f"attT{c}", name=f"attT{c}") for c in range(2)]
                for h in range(NH):
                    up = psA.tile([66, IC], F32, tag="proj", name="U")
                    for grp in groups:
                        g = len(grp)
                        sp = psS.tile([128, 2, IC], F32, tag="S", name="S")
                        for gg, j in enumerate(grp):
                            nc.tensor.matmul(
                                sp[:, gg, :],
                                lhsT=kT[h][:, bass.ts(j, 128)],
                                rhs=qT[h][:, isl],
                                start=True,
                                stop=True,
                            )
                        e = expP.tile([128, 2, IC], BF16, tag="expS", name="expS")
                        nc.scalar.activation(
                            e[:, 0:g, :], sp[:, 0:g, :], Exp, scale=SCALE
                        )
                        for gg, j in enumerate(grp):
                            nc.tensor.matmul(
                                up,
                                lhsT=vA[j][:, h, :],
                                rhs=e[:, gg, :],
                                start=(j == 0),
                                stop=False,
                            )
                    nc.tensor.matmul(
                        up,
                        lhsT=vbar[h],
                        rhs=inval_r[0:1, isl],
                        start=False,
                        stop=True,
                    )
                    rsum = nrm.tile([1, IC], F32, tag="rsum", name="rsum")
                    nc.vector.tensor_copy(rsum, up[64:65, :])
                    rec = nrm.tile([1, IC], F32, tag="rec", name="rec")
                    nc.vector.reciprocal_approx_fast(rec, rsum)
                    bc = nrm.tile([64, IC], F32, tag="bc", name="bc")
                    nc.gpsimd.partition_broadcast(bc, rec[0:1, :])
                    nc.vector.tensor_mul(
                        attT[h // 2][(h % 2) * 64 : (h % 2 + 1) * 64, :],
                        up[0:64, :],
                        bc,
                    )
                if pending is not None:
                    out_proj(*pending)
                pending = (i, attT)
            out_proj(*pending)


_NC_CACHE = {}


def _build(njc):
    key = njc
    if key in _NC_CACHE:
        return _NC_CACHE[key]
    nc = bacc.Bacc("TRN2", debug=False, num_devices=B)
    NKV = njc * 128
    d = {
        "xT": nc.dram_tensor("xT", [DIN, N], BF16, kind="ExternalInput").ap(),
        "xKV": nc.dram_tensor("xKV", [DIN, NKV], BF16, kind="ExternalInput").ap(),
        "Wq": nc.dram_tensor("Wq", [DIN, DM], BF16, kind="ExternalInput").ap(),
        "Wk": nc.dram_tensor("Wk", [DIN, DM], BF16, kind="ExternalInput").ap(),
        "Wv": nc.dram_tensor("Wv", [DIN, DM], BF16, kind="ExternalInput").ap(),
        "Wo": nc.dram_tensor("Wo", [DM, DM], BF16, kind="ExternalInput").ap(),
        "bqk": nc.dram_tensor("bqk", [DM, 2], F32, kind="ExternalInput").ap(),
        "bv": nc.dram_tensor("bv", [1, DM], F32, kind="ExternalInput").ap(),
        "bo": nc.dram_tensor("bo", [DM, 1], F32, kind="ExternalInput").ap(),
        "xbarT": nc.dram_tensor("xbarT", [DIN, 1], BF16, kind="ExternalInput").ap(),
        "qrows": nc.dram_tensor("qrows", [2, N], BF16, kind="ExternalInput").ap(),
        "krows": nc.dram_tensor("krows", [2, NKV], BF16, kind="ExternalInput").ap(),
        "inval": nc.dram_tensor("inval", [1, N], BF16, kind="ExternalInput").ap(),
        "outT": nc.dram_tensor("outT", [DM, N], F32, kind="ExternalOutput").ap(),
    }
    with TileContext(nc) as tc:
        _emit(nc, tc, d, njc)
    nc.compile()
    _NC_CACHE[key] = nc
    return nc


def _host_marshal(x, attention_mask, Wq, bq, Wk, bk, Wv, bv, Wo, bo):
    x = np.asarray(x, dtype=np.float32)
    m = np.asarray(attention_mask).astype(bool)
    pos = np.arange(N)
    start = m.argmax(axis=1)  # first True index
    end = N - 1 - m[:, ::-1].argmax(axis=1)  # last True index (exclusive bound)
    valid = (pos[None, :] >= start[:, None]) & (pos[None, :] < end[:, None])
    valid_f = valid.astype(np.float32)
    vbias_f = np.where(valid, np.float32(0.0), np.float32(NEG)).astype(np.float32)

    A = (start // 128) * 128
    jc = np.ceil(end / 128.0).astype(np.int64) - A // 128
    njc = int(jc.max())
    W = njc * 128

    common = {
        "Wq": np.ascontiguousarray(Wq, dtype=np.float32).astype(NPBF),
        "Wk": np.ascontiguousarray(Wk, dtype=np.float32).astype(NPBF),
        "Wv": np.ascontiguousarray(Wv, dtype=np.float32).astype(NPBF),
        "Wo": np.ascontiguousarray(Wo, dtype=np.float32).astype(NPBF),
        "bqk": np.ascontiguousarray(
            np.stack([np.asarray(bq), np.asarray(bk)], axis=1), dtype=np.float32
        ),
        "bv": np.asarray(bv, dtype=np.float32).reshape(1, DM),
        "bo": np.asarray(bo, dtype=np.float32).reshape(DM, 1),
    }
    in_maps = []
    for b in range(B):
        im = dict(common)
        xTb = np.ascontiguousarray(x[b].T).astype(NPBF)
        im["xT"] = xTb
        a = int(A[b])
        avail = min(N, a + W) - a
        xkv = np.zeros((DIN, W), dtype=NPBF)
        xkv[:, 0:avail] = xTb[:, a : a + avail]
        im["xKV"] = xkv
        im["xbarT"] = x[b].mean(axis=0).reshape(DIN, 1).astype(NPBF)
        inval = np.float32(1.0) - valid_f[b : b + 1]
        im["qrows"] = np.concatenate([valid_f[b : b + 1], inval], axis=0).astype(NPBF)
        kr = np.full((2, W), NEG, dtype=np.float32)
        kr[0, 0:avail] = vbias_f[b, a : a + avail]
        im["krows"] = kr.astype(NPBF)
        im["inval"] = inval.astype(NPBF)
        in_maps.append(im)
    return in_maps, njc


def kernel(x, attention_mask, Wq, bq, Wk, bk, Wv, bv, Wo, bo, _trace=False):
    in_maps, njc = _host_marshal(x, attention_mask, Wq, bq, Wk, bk, Wv, bv, Wo, bo)
    nc = _build(njc)
    res = bass_utils.run_bass_kernel_spmd(
        nc, in_maps, core_ids=list(range(B)), trace=_trace
    )
    out = np.stack([np.ascontiguousarray(r["outT"].T) for r in res.results], axis=0)
    if _trace:
        kernel.last_exec_time_ns = res.exec_time_ns
        kernel.last_results = res
    return out
